# revision 1
# baseline (speedup 1.0000x reference)
"""NCC loss (VoxelMorph-style, 9^3 box window) on 8 Trainium2 NeuronCores.

Strategy: data-parallel over the depth axis. Each core gets a 16-slice output
chunk plus a 4-slice halo on each side (zero-padded at volume edges), for both
batch elements. Per core (box-sum pipeline in bf16, NCC mostly bf16):
  products I*I, J*J (ACT Square), I*J (DVE)
  D-axis win3 stage on DVE; win9 completion via 3 accumulating matmuls
  H-axis then W-axis 9-window sums: chained matmuls against a banded ones
    matrix; lhsT = data (stationary) so each matmul box-sums one axis AND
    transposes, landing back in [H', W'] layout with no transpose insts
  per-G-slice groups: PSUM drain + NCC elementwise math + partial reduction,
    pipelined across groups/volumes to keep DVE/ACT/PE all busy
Host sums the 8x128 partials and forms 1 - total/N.
"""

from contextlib import ExitStack

import numpy as np

WIN = 9
PAD = WIN // 2  # 4
B = 2
D = 128
H = 128
W = 128
NCORES = 8
D_OUT = D // NCORES  # 16
D_IN = D_OUT + 2 * PAD  # 24
EPS = 1e-6
WIN_SIZE = 729.0
N_TOTAL = float(B * D * H * W)

_CACHE = {}


def _split_multiwaits(nc):
    """Walrus in this env encodes at most ONE sync-wait per instruction.
    Hoist extra waits onto standalone EventSemaphore insts just before."""
    from concourse import mybir

    n = 0
    for fn in nc.m.functions:
        for bb in fn.blocks:
            il = bb.instructions
            out = []
            for inst in il:
                si = inst.sync_info
                if si is not None and si.on_wait and len(si.on_wait) > 1:
                    waits = list(si.on_wait)
                    for w in waits[:-1]:
                        ev = mybir.InstEventSemaphore(
                            name=f"EVW-{n}", ins=[], outs=[])
                        n += 1
                        ev.engine = inst.engine
                        ev.sync_info = mybir.SyncInfo(on_wait=[w],
                                                      on_update=[])
                        out.append(ev)
                    inst.sync_info = mybir.SyncInfo(
                        on_wait=[waits[-1]], on_update=list(si.on_update))
                out.append(inst)
            il[:] = out
    return n


def _build_nc():
    import concourse.bass as bass
    import concourse.tile as tile
    from concourse import mybir

    f32 = mybir.dt.float32
    bf16 = mybir.dt.bfloat16
    Alu = mybir.AluOpType
    Act = mybir.ActivationFunctionType

    nc = bass.Bass()
    I_ext = nc.declare_dram_parameter("I", [B, D_IN, H, W], bf16,
                                      isOutput=False)
    J_ext = nc.declare_dram_parameter("J", [B, D_IN, H, W], bf16,
                                      isOutput=False)
    BB_ext = nc.declare_dram_parameter("BB", [H, H], bf16, isOutput=False)
    out_ext = nc.declare_dram_parameter("partials", [128, 1], f32,
                                        isOutput=True)

    G = 8  # slices per PSUM drain / NCC group
    NV = D_IN - 2  # 22 win3 slices
    VOLS = ("I", "J", "I2", "J2", "IJ")

    with tile.TileContext(nc) as tc, ExitStack() as ctx:
        singles = ctx.enter_context(tc.tile_pool(name="singles", bufs=1))
        src = ctx.enter_context(tc.tile_pool(name="src", bufs=2))
        dtmp = ctx.enter_context(tc.tile_pool(name="dtmp", bufs=2))
        boxp = ctx.enter_context(tc.tile_pool(name="boxp", bufs=2))
        yp = ctx.enter_context(tc.tile_pool(name="yp", bufs=3))
        qp = ctx.enter_context(tc.tile_pool(name="qp", bufs=3))
        pp = ctx.enter_context(tc.tile_pool(name="pp", bufs=8))
        psum = ctx.enter_context(tc.tile_pool(name="psum", bufs=2,
                                              space="PSUM"))

        BBt = singles.tile([H, H], bf16)
        nc.sync.dma_start(out=BBt, in_=BB_ext[:, :])

        # prefetch both batches' inputs up front (SP program order = early)
        tIbs, tJbs = [], []
        for b in range(B):
            tIb = src.tile([H, D_IN, W], bf16, tag="tIb")
            tJb = src.tile([H, D_IN, W], bf16, tag="tJb")
            nc.sync.dma_start(out=tIb,
                              in_=I_ext[b].rearrange("d h w -> h d w"))
            nc.sync.dma_start(out=tJb,
                              in_=J_ext[b].rearrange("d h w -> h d w"))
            tIbs.append(tIb)
            tJbs.append(tJb)

        parts = []
        for b in range(B):
            tIb, tJb = tIbs[b], tJbs[b]

            tI2 = src.tile([H, D_IN, W], bf16, tag="tI2")
            tJ2 = src.tile([H, D_IN, W], bf16, tag="tJ2")
            tIJ = src.tile([H, D_IN, W], bf16, tag="tIJ")
            nc.scalar.activation(out=tI2, in_=tIb, func=Act.Square)
            nc.scalar.activation(out=tJ2, in_=tJb, func=Act.Square)
            nc.vector.tensor_mul(out=tIJ, in0=tIb, in1=tJb)

            # D-axis win3 for all 5 volumes (bf16 adds on DVE, flat 2D APs
            # so the 2x perf mode engages)
            t3s = {}
            for name, V in (("I", tIb), ("J", tJb), ("I2", tI2),
                            ("J2", tJ2), ("IJ", tIJ)):
                Vf = V.rearrange("p d w -> p (d w)")
                a = dtmp.tile([H, NV, W], bf16, tag="a")
                t3 = dtmp.tile([H, NV, W], bf16, tag="t3" + name)
                af = a.rearrange("p d w -> p (d w)")
                t3f = t3.rearrange("p d w -> p (d w)")
                n = NV * W
                nc.vector.tensor_add(out=af, in0=Vf[:, 0:n],
                                     in1=Vf[:, W:W + n])
                nc.vector.tensor_add(out=t3f, in0=af,
                                     in1=Vf[:, 2 * W:2 * W + n])
                t3s[name] = t3

            # per-group: H+W matmul passes for all 5 vols, then NCC + reduce.
            # Final batch uses smaller tail groups to shrink the exposed
            # serial NCC chain at the end of the kernel.
            groups = [(0, G), (G, G)] if b == 0 else [(0, G), (G, G // 2),
                                                     (G + G // 2, G // 2)]
            for g, gn in groups:
                boxg = {}
                for name in VOLS:
                    t3 = t3s[name]
                    pg1 = psum.tile([128, G, 128], f32, tag="pg1")
                    for k in range(gn):
                        for mi, m in enumerate((0, 3, 6)):
                            nc.tensor.matmul(out=pg1[:, k, :],
                                             lhsT=t3[:, g + k + m, :],
                                             rhs=BBt, start=(mi == 0),
                                             stop=(mi == 2))
                    y = yp.tile([128, G, 128], bf16, tag="y")
                    nc.scalar.copy(out=y.rearrange("p a b -> p (a b)")
                                   [:, :gn * 128],
                                   in_=pg1.rearrange("p a b -> p (a b)")
                                   [:, :gn * 128])
                    pg2 = psum.tile([128, G, 128], f32, tag="pg2")
                    for k in range(gn):
                        nc.tensor.matmul(out=pg2[:, k, :], lhsT=y[:, k, :],
                                         rhs=BBt, start=True, stop=True)
                    bx = boxp.tile([128, G * W], bf16, tag="bx" + name)
                    cp = nc.scalar.copy if name in ("I", "J") else \
                        nc.vector.tensor_copy
                    cp(out=bx[:, :gn * 128],
                       in_=pg2.rearrange("p a b -> p (a b)")[:, :gn * 128])
                    boxg[name] = bx[:, :gn * 128]

                SI, SJ, SI2, SJ2, SIJ = (boxg[k] for k in VOLS)
                qa_t = qp.tile([128, G * W], bf16, tag="qa")
                qb_t = qp.tile([128, G * W], bf16, tag="qb")
                qc_t = qp.tile([128, G * W], bf16, tag="qc")
                qd_t = qp.tile([128, G * W], bf16, tag="qd")
                qa = qa_t[:, :gn * 128]
                qb = qb_t[:, :gn * 128]
                qc = qc_t[:, :gn * 128]
                qd = qd_t[:, :gn * 128]
                # q = (S/27)^2 ; var = S2 - q (clamped to eps)
                nc.scalar.activation(out=qa, in_=SI, func=Act.Square,
                                     scale=1.0 / 27.0)
                nc.scalar.activation(out=qb, in_=SJ, func=Act.Square,
                                     scale=1.0 / 27.0)
                nc.vector.tensor_sub(out=qc, in0=SI2, in1=qa)
                nc.vector.tensor_sub(out=qd, in0=SJ2, in1=qb)
                nc.vector.tensor_scalar_max(out=qc, in0=qc, scalar1=EPS)
                nc.vector.tensor_scalar_max(out=qd, in0=qd, scalar1=EPS)
                nc.vector.tensor_mul(out=qa, in0=qc, in1=qd)
                # r = rsqrt(V)/729 = exp(-0.5 * ln(V * 729^2))  (ACT)
                nc.scalar.activation(out=qb, in_=qa, func=Act.Ln,
                                     scale=WIN_SIZE * WIN_SIZE)
                nc.scalar.activation(out=qa, in_=qb, func=Act.Exp,
                                     scale=-0.5)
                # C9 = 729*IJ_sum - I_sum*J_sum ; cc = C9 * r (+ fused sum)
                nc.vector.tensor_mul(out=qc, in0=SI, in1=SJ)
                nc.vector.tensor_scalar_mul(out=qd, in0=SIJ,
                                            scalar1=WIN_SIZE)
                nc.vector.tensor_sub(out=qb, in0=qd, in1=qc)
                part = pp.tile([128, 1], f32, tag="part")
                nc.vector.scalar_tensor_tensor(out=qc, in0=qb, scalar=0.0,
                                               in1=qa, op0=Alu.add,
                                               op1=Alu.mult,
                                               accum_out=part)
                parts.append(part)

        # combine the group partials
        tA = pp.tile([128, 1], f32, tag="tA")
        tB = pp.tile([128, 1], f32, tag="tB")
        total = pp.tile([128, 1], f32, tag="total")
        nc.vector.tensor_add(out=tA, in0=parts[0], in1=parts[1])
        nc.vector.tensor_add(out=tB, in0=parts[2], in1=parts[3])
        nc.vector.tensor_add(out=tB, in0=tB, in1=parts[4])
        nc.vector.tensor_add(out=total, in0=tA, in1=tB)
        nc.sync.dma_start(out=out_ext[:, :], in_=total)

    return nc


def _get_nc(split=True):
    if "nc" not in _CACHE:
        _CACHE["nc"] = _build_nc()
    if split and not _CACHE.get("split"):
        _split_multiwaits(_CACHE["nc"])
        _CACHE["split"] = True
    return _CACHE["nc"]


def _shards(y_true, y_pred):
    import ml_dtypes

    yt = np.ascontiguousarray(
        np.asarray(y_true, dtype=np.float32).reshape(B, D, H, W))
    yp = np.ascontiguousarray(
        np.asarray(y_pred, dtype=np.float32).reshape(B, D, H, W))
    pt = np.zeros((B, D + 2 * PAD, H, W), dtype=ml_dtypes.bfloat16)
    pp = np.zeros((B, D + 2 * PAD, H, W), dtype=ml_dtypes.bfloat16)
    pt[:, PAD:PAD + D] = yt.astype(ml_dtypes.bfloat16)
    pp[:, PAD:PAD + D] = yp.astype(ml_dtypes.bfloat16)

    BB = np.zeros((H, H), dtype=np.float32)
    for i in range(H):
        BB[i, max(0, i - PAD):min(H, i + PAD + 1)] = 1.0
    BB_bf16 = BB.astype(ml_dtypes.bfloat16)

    in_maps = []
    for c in range(NCORES):
        lo = c * D_OUT
        in_maps.append({
            "I": np.ascontiguousarray(pt[:, lo:lo + D_IN]),
            "J": np.ascontiguousarray(pp[:, lo:lo + D_IN]),
            "BB": BB_bf16,
        })
    return in_maps


def run(y_true, y_pred, trace=False):
    from concourse.bass_utils import run_bass_kernel_spmd

    nc = _get_nc()
    in_maps = _shards(y_true, y_pred)
    res = run_bass_kernel_spmd(nc, in_maps, list(range(NCORES)), trace=trace)
    total = 0.0
    for r in res.results:
        total += float(np.asarray(r["partials"], dtype=np.float64).sum())
    loss = np.float32(1.0 - total / N_TOTAL)
    return np.array(loss, dtype=np.float32), res


def kernel(y_true, y_pred):
    loss, _ = run(y_true, y_pred, trace=False)
    return loss



# revision 4
# speedup vs baseline: 1.1598x; 1.1598x over previous
"""NCC loss (VoxelMorph-style, 9^3 box window) on 8 Trainium2 NeuronCores.

Data-parallel over depth: each core handles 16 output slices (+4-slice halos)
for both batch elements.  Per core, for each of 5 volumes (I, J, I*J, I^2, J^2):

  win3-D (stride-3):  s[d] = x[d] + x[d+3] + x[d+6]          (DVE, bf16 2x)
  pass A:  per s-slice matmul, lhsT = data (stationary), rhs = banded-ones BB
           -> box-sums the H axis AND transposes to [W, H'] in PSUM   (PE)
  drain:   PSUM -> SBUF bf16 y tiles                          (ACT copies)
  pass B:  lhsT = BB (stationary, no per-MM weight reload), rhs = y streamed
           512 wide; 3 accumulating MMs with d-offsets 0,1,2 complete the
           win9-D sum while box-summing W -> full box sums in PSUM    (PE)
  NCC:     per 4-slice group, read the 5 box-sum banks straight from PSUM:
           A = 729*I2s - SI^2, B = 729*J2s - SJ^2, C = 729*IJs - SI*SJ,
           cc = C * exp(-0.5*ln(A*B)); fused reduce into per-group partials.

Host sums the 8 x [128 x 8] partials and forms 1 - total/N.
"""

from contextlib import ExitStack

import numpy as np

WIN = 9
PAD = WIN // 2  # 4
B = 2
D = 128
H = 128
W = 128
NCORES = 8
D_OUT = D // NCORES  # 16
D_IN = D_OUT + 2 * PAD  # 24
NS = D_IN - 6  # 18 stride-3 win3 slices
WIN_SIZE = 729.0
N_TOTAL = float(B * D * H * W)
NG = 4  # 4-slice groups per batch
GS = D_OUT // NG  # 4 slices per group

_CACHE = {}


def _split_multiwaits(nc):
    """Walrus in this env encodes at most ONE sync-wait per instruction.
    Hoist extra waits onto standalone EventSemaphore insts just before."""
    from concourse import mybir

    n = 0
    for fn in nc.m.functions:
        for bb in fn.blocks:
            il = bb.instructions
            out = []
            for inst in il:
                si = inst.sync_info
                if si is not None and si.on_wait and len(si.on_wait) > 1:
                    waits = list(si.on_wait)
                    for w in waits[:-1]:
                        ev = mybir.InstEventSemaphore(
                            name=f"EVW-{n}", ins=[], outs=[])
                        n += 1
                        ev.engine = inst.engine
                        ev.sync_info = mybir.SyncInfo(on_wait=[w],
                                                      on_update=[])
                        out.append(ev)
                    inst.sync_info = mybir.SyncInfo(
                        on_wait=[waits[-1]], on_update=list(si.on_update))
                out.append(inst)
            il[:] = out
    return n


VOLS = ("I", "J", "IJ", "I2", "J2")
CHUNKS = ((0, 4), (4, 4), (8, 4), (12, 4), (16, 2))  # pass-A psum chunks


def _build_nc():
    import concourse.bass as bass
    import concourse.tile as tile
    from concourse import mybir

    f32 = mybir.dt.float32
    bf16 = mybir.dt.bfloat16
    Alu = mybir.AluOpType
    Act = mybir.ActivationFunctionType

    nc = bass.Bass()
    I_ext = nc.declare_dram_parameter("I", [B, H, D_IN * W], bf16,
                                      isOutput=False)
    J_ext = nc.declare_dram_parameter("J", [B, H, D_IN * W], bf16,
                                      isOutput=False)
    BB_ext = nc.declare_dram_parameter("BB", [H, H], bf16, isOutput=False)
    out_ext = nc.declare_dram_parameter("partials", [128, B * NG], f32,
                                        isOutput=True)

    with tile.TileContext(nc) as tc, ExitStack() as ctx:
        const = ctx.enter_context(tc.tile_pool(name="const", bufs=1))
        src = ctx.enter_context(tc.tile_pool(name="src", bufs=2))
        prod = ctx.enter_context(tc.tile_pool(name="prod", bufs=2))
        wtmp = ctx.enter_context(tc.tile_pool(name="wtmp", bufs=2))
        sp = ctx.enter_context(tc.tile_pool(name="sp", bufs=4))
        yp = ctx.enter_context(tc.tile_pool(name="yp", bufs=6))
        nccf = ctx.enter_context(tc.tile_pool(name="nccf", bufs=2))
        nccb = ctx.enter_context(tc.tile_pool(name="nccb", bufs=2))
        pout = ctx.enter_context(tc.tile_pool(name="pout", bufs=1))
        pa = ctx.enter_context(tc.tile_pool(name="pa", bufs=2, space="PSUM"))
        pb = ctx.enter_context(tc.tile_pool(name="pb", bufs=6, space="PSUM"))

        BBt = const.tile([H, H], bf16)
        nc.sync.dma_start(out=BBt, in_=BB_ext[:, :])
        partsT = pout.tile([128, B * NG], f32)

        # input DMAs (contiguous: host pre-transposed to [B, H, D_IN*W])
        tI, tJ = [], []
        for b in range(B):
            ti = src.tile([H, D_IN * W], bf16, tag="tI")
            tj = src.tile([H, D_IN * W], bf16, tag="tJ")
            nc.sync.dma_start(out=ti, in_=I_ext[b])
            nc.sync.dma_start(out=tj, in_=J_ext[b])
            tI.append(ti)
            tJ.append(tj)

        # products + win3-D (stride-3) for all volumes, both batches
        n3 = NS * W

        def win3(x):
            a = wtmp.tile([H, NS * W], bf16, tag="a")
            s = sp.tile([H, NS * W], bf16, tag="s")
            nc.vector.tensor_add(out=a, in0=x[:, 0:n3],
                                 in1=x[:, 3 * W:3 * W + n3])
            nc.vector.tensor_add(out=s, in0=a, in1=x[:, 6 * W:6 * W + n3])
            return s

        svols = [{} for _ in range(B)]
        for b in range(B):
            ti2 = prod.tile([H, D_IN * W], bf16, tag="tI2")
            tj2 = prod.tile([H, D_IN * W], bf16, tag="tJ2")
            tij = prod.tile([H, D_IN * W], bf16, tag="tIJ")
            nc.scalar.activation(out=ti2, in_=tI[b], func=Act.Square)
            svols[b]["I"] = win3(tI[b])
            nc.scalar.activation(out=tj2, in_=tJ[b], func=Act.Square)
            nc.vector.tensor_mul(out=tij, in0=tI[b], in1=tJ[b])
            svols[b]["J"] = win3(tJ[b])
            svols[b]["IJ"] = win3(tij)
            svols[b]["I2"] = win3(ti2)
            svols[b]["J2"] = win3(tj2)

        yvols = [{} for _ in range(B)]
        pbt = [[{} for _ in range(NG)] for _ in range(B)]

        def pass_ab(b):
            # pass A + drains, per volume
            for v in VOLS:
                s = svols[b][v]
                y = yp.tile([H, NS * W], bf16, tag="y")
                yvols[b][v] = y
                for k0, kn in CHUNKS:
                    pat = pa.tile([128, 512], f32, tag="pa")
                    for j in range(kn):
                        k = k0 + j
                        nc.tensor.matmul(out=pat[:, j * 128:(j + 1) * 128],
                                         lhsT=s[:, k * W:(k + 1) * W],
                                         rhs=BBt, start=True, stop=True)
                    nc.scalar.copy(out=y[:, k0 * W:(k0 + kn) * W],
                                   in_=pat[:, :kn * 128])
            # pass B: BB stationary, stream y 512-wide, 3 accumulating MMs
            for g in range(NG):
                d0 = g * GS
                for v in VOLS:
                    t = pb.tile([128, GS * W], f32, tag="pb")
                    pbt[b][g][v] = t
                    y = yvols[b][v]
                    for m in range(3):
                        nc.tensor.matmul(
                            out=t,
                            lhsT=BBt,
                            rhs=y[:, (d0 + m) * W:(d0 + m + GS) * W],
                            start=(m == 0), stop=(m == 2))

        def ncc_pre(b):
            # SI/27, SJ/27 -> SBUF bf16 (ACT), then their products (DVE, 2x)
            for g in range(NG):
                t = pbt[b][g]
                si = nccb.tile([128, 512], bf16, tag="si")
                sj = nccb.tile([128, 512], bf16, tag="sj")
                nc.scalar.activation(out=si, in_=t["I"], func=Act.Copy,
                                     scale=1.0 / 27.0)
                nc.scalar.activation(out=sj, in_=t["J"], func=Act.Copy,
                                     scale=1.0 / 27.0)
                sa = nccb.tile([128, 512], bf16, tag="sa")
                sb = nccb.tile([128, 512], bf16, tag="sb")
                sc = nccb.tile([128, 512], bf16, tag="sc")
                nc.vector.tensor_mul(out=sa, in0=si, in1=si)
                nc.vector.tensor_mul(out=sb, in0=sj, in1=sj)
                nc.vector.tensor_mul(out=sc, in0=si, in1=sj)
                t["sa"], t["sb"], t["sc"] = sa, sb, sc

        def ncc_main(b):
            # working in the /729 domain: A' = I2s - (SI/27)^2 = I_var, etc.
            # cc = (IJs - SI*SJ/729) * exp(-0.5*ln(A'*B'))
            for g in range(NG):
                t = pbt[b][g]
                va = nccb.tile([128, 512], bf16, tag="va")
                vb = nccb.tile([128, 512], bf16, tag="vb")
                vc = nccb.tile([128, 512], bf16, tag="vc")
                p = nccb.tile([128, 512], bf16, tag="p")
                q = nccb.tile([128, 512], bf16, tag="q")
                r = nccb.tile([128, 512], bf16, tag="r")
                fin = nccb.tile([128, 512], bf16, tag="fin")
                nc.vector.tensor_sub(out=va, in0=t["I2"], in1=t["sa"])
                nc.vector.tensor_sub(out=vb, in0=t["J2"], in1=t["sb"])
                nc.vector.tensor_sub(out=vc, in0=t["IJ"], in1=t["sc"])
                nc.vector.tensor_mul(out=p, in0=va, in1=vb)
                nc.scalar.activation(out=q, in_=p, func=Act.Ln)
                nc.scalar.activation(out=r, in_=q, func=Act.Exp, scale=-0.5)
                gi = b * NG + g
                nc.vector.scalar_tensor_tensor(
                    out=fin, in0=vc, scalar=0.0, in1=r,
                    op0=Alu.add, op1=Alu.mult,
                    accum_out=partsT[:, gi:gi + 1])

        pass_ab(0)
        ncc_pre(0)
        pass_ab(1)
        ncc_main(0)
        ncc_pre(1)
        ncc_main(1)

        nc.sync.dma_start(out=out_ext[:, :], in_=partsT)

    return nc


def _get_nc(split=True):
    if "nc" not in _CACHE:
        _CACHE["nc"] = _build_nc()
    if split and not _CACHE.get("split"):
        _split_multiwaits(_CACHE["nc"])
        _CACHE["split"] = True
    return _CACHE["nc"]


def _shards(y_true, y_pred):
    import ml_dtypes

    yt = np.ascontiguousarray(
        np.asarray(y_true, dtype=np.float32).reshape(B, D, H, W))
    yp = np.ascontiguousarray(
        np.asarray(y_pred, dtype=np.float32).reshape(B, D, H, W))
    pt = np.zeros((B, D + 2 * PAD, H, W), dtype=ml_dtypes.bfloat16)
    pp = np.zeros((B, D + 2 * PAD, H, W), dtype=ml_dtypes.bfloat16)
    pt[:, PAD:PAD + D] = yt.astype(ml_dtypes.bfloat16)
    pp[:, PAD:PAD + D] = yp.astype(ml_dtypes.bfloat16)

    BB = np.zeros((H, H), dtype=np.float32)
    for i in range(H):
        BB[i, max(0, i - PAD):min(H, i + PAD + 1)] = 1.0
    BB_bf16 = BB.astype(ml_dtypes.bfloat16)

    in_maps = []
    for c in range(NCORES):
        lo = c * D_OUT
        # transpose to [B, H, D_IN, W] so the on-device layout (partition=H)
        # is a fully contiguous DMA
        icore = np.ascontiguousarray(
            pt[:, lo:lo + D_IN].transpose(0, 2, 1, 3)).reshape(B, H, -1)
        jcore = np.ascontiguousarray(
            pp[:, lo:lo + D_IN].transpose(0, 2, 1, 3)).reshape(B, H, -1)
        in_maps.append({"I": icore, "J": jcore, "BB": BB_bf16})
    return in_maps


def run(y_true, y_pred, trace=False):
    from concourse.bass_utils import run_bass_kernel_spmd

    nc = _get_nc()
    in_maps = _shards(y_true, y_pred)
    res = run_bass_kernel_spmd(nc, in_maps, list(range(NCORES)), trace=trace)
    total = 0.0
    for r in res.results:
        total += float(np.asarray(r["partials"], dtype=np.float64).sum())
    loss = np.float32(1.0 - total / N_TOTAL)
    return np.array(loss, dtype=np.float32), res


def kernel(y_true, y_pred):
    loss, _ = run(y_true, y_pred, trace=False)
    return loss
